# revision 50
# baseline (speedup 1.0000x reference)
"""Trainium2 Bass kernel for nn_Center2D (DWT -> pool -> conv-BN-ReLU x2 -> deconv -> IDWT).

Self-contained: hardcodes shapes from the problem spec.
Sharding: pure data parallel, batch dim (8) across 8 cores; BN batch stats
synchronized with a tiny AllReduce (2x128 floats) per BN layer.

Layout strategy per core (one sample):
  front: x loaded contiguously as [(h-half, c) -> 128 partitions, 128 h, 256 w]
         (32KB descriptors, sprays across all 16 SDMA engines), DWT-W and
         DWT-H as 4-tap FIR on DVE (strided slices along free dims), pool on
         DVE, one small SBUF->SBUF DMA to consolidate h-halves for conv1,
  mid:   conv1 as 9 matmuls (64-part contraction), conv2 as 9 K-packed PE
         matmuls per output chunk, BN stats via accum_out during PSUM
         evacuation, tiny AllReduce (warmed up by a dummy collective at t=0),
         BN+ReLU fused into one ACT op,
  back:  deconv as 4 PE matmuls, DRAM round-trip to put H on partitions,
         PE matmul for IDWT-H (banded matrix BH), DVE taps for final IDWT-W,
         output stores split across 16 ExternalOutput tensors (stores to one
         ExternalOutput pin all descriptors of a call onto one SDMA engine).
"""

import os
import numpy as np
import ml_dtypes

import concourse.bass as bass
import concourse.bacc as bacc
import concourse.tile as tile
from concourse import mybir
from concourse.bass_utils import run_bass_kernel_spmd

F32 = mybir.dt.float32
# bf16 (not fp16): the PE streams 2 cols/cycle for bf16 but 1 for fp16 on
# 128-row contractions; DVE speed is identical, and accuracy stays well
# under the 2e-2 gate
F16 = mybir.dt.bfloat16
AF = mybir.ActivationFunctionType
ALU = mybir.AluOpType

REC = np.array([0.48296291314469025, 0.8365163037378079,
                0.22414386804185735, -0.12940952255092145], dtype=np.float64)
DEC = REC[::-1].copy()

N_CORES = int(os.environ.get("WK_CORES", "8"))
EPS = 1e-5


# ---------------------------------------------------------------- host consts
def build_BH():
    """IDWT along one axis as a dense [128, 254] matrix."""
    B = np.zeros((128, 254), dtype=np.float64)
    for t in range(127):
        B[t,   2*t] += REC[2]
        B[t+1, 2*t] += REC[0]
        B[t,   2*t+1] += REC[3]
        B[t+1, 2*t+1] += REC[1]
    return B.astype(np.float32)


def pack_consts(conv1_w, conv2_w, deconv_w, deconv_b, bn1_g, bn1_b, bn2_g, bn2_b):
    bhw = build_BH().astype(ml_dtypes.bfloat16)          # [128, 254]

    w1t = np.zeros((64, 9 * 128), ml_dtypes.bfloat16)    # rows ci, cols (ky,kx,co)
    for ky in range(3):
        for kx in range(3):
            w1t[:, (ky*3+kx)*128:(ky*3+kx+1)*128] = conv1_w[:, :, ky, kx].T

    w2t = np.zeros((128, 9 * 128), ml_dtypes.bfloat16)
    for ky in range(3):
        for kx in range(3):
            w2t[:, (ky*3+kx)*128:(ky*3+kx+1)*128] = conv2_w[:, :, ky, kx].T

    wdt = np.zeros((128, 4 * 64), ml_dtypes.bfloat16)    # [ci, (k,l,o)]
    for k in range(2):
        for l in range(2):
            wdt[:, (k*2+l)*64:(k*2+l+1)*64] = deconv_w[:, :, k, l]

    return {
        "BHW": bhw,
        "w1t": w1t,
        "w2t": w2t,
        "wdt": wdt,
        "db": deconv_b.reshape(64, 1).astype(np.float32),
        "bn1g": bn1_g.reshape(128, 1).astype(np.float32),
        "bn1b": bn1_b.reshape(128, 1).astype(np.float32),
        "bn2g": bn2_g.reshape(128, 1).astype(np.float32),
        "bn2b": bn2_b.reshape(128, 1).astype(np.float32),
    }


# ---------------------------------------------------------------- bass kernel
def build_nc(world=N_CORES, stage=None):
    if stage is None:
        stage = int(os.environ.get("WK_STAGE", "99"))
    nc = bacc.Bacc("TRN2", target_bir_lowering=False)
    use_cc = world > 1

    x = nc.dram_tensor("x", (64, 256, 256), F32, kind="ExternalInput")
    bhw_d = nc.dram_tensor("BHW", (128, 254), F16, kind="ExternalInput")
    w1t_d = nc.dram_tensor("w1t", (64, 1152), F16, kind="ExternalInput")
    w2t_d = nc.dram_tensor("w2t", (128, 1152), F16, kind="ExternalInput")
    wdt_d = nc.dram_tensor("wdt", (128, 256), F16, kind="ExternalInput")
    db_d = nc.dram_tensor("db", (64, 1), F32, kind="ExternalInput")
    bn_vecs = {n: nc.dram_tensor(n, (128, 1), F32, kind="ExternalInput")
               for n in ("bn1g", "bn1b", "bn2g", "bn2b")}
    # 16 separate output tensors (h-slices): writes to an ExternalOutput pin
    # all descriptors of a call onto one SDMA engine, so split into 16 calls
    OUT_SPLITS = []
    h0 = 0
    for i in range(16):
        sz = 16 if (i % 8) != 7 else 15
        OUT_SPLITS.append((h0, sz))
        h0 += sz
    out_ds = [nc.dram_tensor(f"out{i}", (sz, 64, 254), F32, kind="ExternalOutput")
              for i, (_, sz) in enumerate(OUT_SPLITS)]

    scr2 = nc.dram_tensor("scr2", (64, 128, 128), F16, kind="Internal")
    cc_bufs = []
    for i in (0, 1, 2):
        cc_bufs.append((
            nc.dram_tensor(f"bn{i}_in", (128, 2), F32, kind="Internal"),
            nc.dram_tensor(f"bn{i}_out", (128, 2), F32, kind="Internal",
                           addr_space="Shared"),
        ))
    rg = [list(range(world))]
    cnt = float(world * 64 * 64)

    with tile.TileContext(nc) as tc, \
         tc.tile_pool(name="persist", bufs=1) as pp:
        def _body():
            # warmup collective: absorbs the ~11us first-call ncfw setup so
            # the BN1 AllReduce doesn't pay it; runs under the x load
            if use_cc:
                nc.gpsimd.collective_compute(
                    "AllReduce", ALU.add, replica_groups=rg,
                    ins=[cc_bufs[0][0][:]], outs=[cc_bufs[0][1][:]])

            # ---------------- x chunk 0 load first: it heads the whole
            # dependency graph, so it must not queue behind the const DMAs
            xg = x[:].rearrange("c (g r) w -> g c (r w)", g=2)  # [2,64,128*256]
            xin_pool = tc.tile_pool(name="xin", bufs=2)
            xin = xin_pool.__enter__()
            xc0 = xin.tile([128, 32 * 256], F32, tag="xc")
            nc.sync.dma_start(xc0[0:64], xg[0, :, 0:8192])
            nc.scalar.dma_start(xc0[64:128], xg[1, :, 0:8192])

            # ---------------- consts to SBUF
            bhw_sb = pp.tile([128, 254], F16, name="bhw_sb")
            nc.sync.dma_start(bhw_sb[:], bhw_d[:])
            w1t_sb = pp.tile([64, 1152], F16, name="w1t_sb")
            nc.sync.dma_start(w1t_sb[:], w1t_d[:])
            w2t_sb = pp.tile([128, 1152], F16, name="w2t_sb")
            nc.sync.dma_start(w2t_sb[:], w2t_d[:])
            wdt_sb = pp.tile([128, 256], F16, name="wdt_sb")
            nc.sync.dma_start(wdt_sb[:], wdt_d[:])
            db_sb = pp.tile([64, 1], F32, name="db_sb")
            nc.sync.dma_start(db_sb[:], db_d[:])
            bnv = {}
            for n, d in bn_vecs.items():
                t = pp.tile([128, 1], F32, name=f"{n}_sb")
                nc.sync.dma_start(t[:], d[:])
                bnv[n] = t

            # conv1 input (lives past the front scope), 64 partitions
            in1_pad = pp.tile([64, 66 * 66], F16, name="in1_pad")
            nc.vector.memset(in1_pad[:], 0.0)
            p1v = in1_pad[:].rearrange("p (r v) -> p r v", v=66)

            d0, d1, d2, d3 = (float(DEC[0]), float(DEC[1]),
                              float(DEC[2]), float(DEC[3]))

            # ---------------- front: all-DVE DWT + pool
            # x viewed as [(g c) -> 128 partitions, h_local, w]; partition
            # group g=0 holds h 0:128, g=1 holds h 128:256 of channel c
            front_pool = tc.tile_pool(name="front", bufs=1)
            fp = front_pool.__enter__()

            y_t = fp.tile([128, 128 * 128], F16, name="y_t")    # DWT-W out
            y_v = y_t[:].rearrange("p (h t) -> p h t", t=128)
            y2_t = fp.tile([128, 64 * 128], F16, name="y2_t")   # DWT-H out
            y2_v = y2_t[:].rearrange("p (s t) -> p s t", t=128)
            pw_t = fp.tile([128, 64 * 64], F16, name="pw_t")    # pool-W out
            pw_v = pw_t[:].rearrange("p (s u) -> p s u", u=64)
            pg1 = fp.tile([128, 32 * 64], F16, name="pg1")      # pool-H grp 1
            pg1_v = pg1[:].rearrange("p (s u) -> p s u", u=64)

            def dwt_h_piece(a, b, th_p):
                """y2[s] = d3 y[2s-2] + d2 y[2s-1] + d1 y[2s] + d0 y[2s+1],
                s_local in [a, b) (a >= 1), both partition groups at once."""
                tv2 = th_p[:].rearrange("p (s t) -> p s t", t=128)[:, 0:b-a, :]
                nc.vector.tensor_scalar(tv2, y_v[:, 2*a-2:2*b-3:2, :], d3,
                                        None, ALU.mult)
                nc.vector.scalar_tensor_tensor(tv2, y_v[:, 2*a-1:2*b-2:2, :],
                                               d2, tv2, ALU.mult, ALU.add)
                nc.vector.scalar_tensor_tensor(tv2, y_v[:, 2*a:2*b-1:2, :],
                                               d1, tv2, ALU.mult, ALU.add)
                nc.vector.scalar_tensor_tensor(y2_v[:, a:b, :],
                                               y_v[:, 2*a+1:2*b:2, :],
                                               d0, tv2, ALU.mult, ALU.add)

            with tc.tile_pool(name="twp", bufs=2) as twp, \
                 tc.tile_pool(name="thp", bufs=2) as thp:
                for hc in range(4):          # h-chunks of 32 rows
                    if hc == 0:
                        xc = xc0
                    else:
                        xc = xin.tile([128, 32 * 256], F32, tag="xc")
                        # one call per h-half: a [64, 8192] 2-dim AP sprays
                        # per-partition; a 3-dim [2, 64, 8192] sprays by dim0
                        # and serializes onto 2 engines
                        nc.sync.dma_start(xc[0:64], xg[0, :, hc*8192:(hc+1)*8192])
                        nc.scalar.dma_start(xc[64:128],
                                            xg[1, :, hc*8192:(hc+1)*8192])
                    xv = xc[:].rearrange("p (h w) -> p h w", w=256)
                    yc = y_v[:, hc*32:(hc+1)*32, :]
                    tw = twp.tile([128, 32 * 127], F16, tag="tw")
                    tv = tw[:].rearrange("p (h t) -> p h t", t=127)
                    # DWT-W main taps t=1..127
                    nc.vector.tensor_scalar(tv, xv[:, :, 0:253:2], d3, None,
                                            ALU.mult)
                    nc.vector.scalar_tensor_tensor(tv, xv[:, :, 1:254:2], d2,
                                                   tv, ALU.mult, ALU.add)
                    nc.vector.scalar_tensor_tensor(tv, xv[:, :, 2:255:2], d1,
                                                   tv, ALU.mult, ALU.add)
                    nc.vector.scalar_tensor_tensor(yc[:, :, 1:128],
                                                   xv[:, :, 3:256:2], d0,
                                                   tv, ALU.mult, ALU.add)
                    # t=0 mirror: (d1+d2)*x0 + (d0+d3)*x1
                    nc.vector.tensor_scalar(yc[:, :, 0:1], xv[:, :, 1:2],
                                            d0 + d3, None, ALU.mult)
                    nc.vector.scalar_tensor_tensor(yc[:, :, 0:1],
                                                   xv[:, :, 0:1], d1 + d2,
                                                   yc[:, :, 0:1],
                                                   ALU.mult, ALU.add)
                    # DWT-H piece for the s-range this chunk completes
                    # (chunk hc covers h <= 32*hc+31 -> s <= 16*hc+15),
                    # then pool-W for those s rows; both hide under the
                    # next chunk's load
                    a = max(1, 16 * hc)
                    b = 16 * hc + 16
                    th_p = thp.tile([128, 16 * 128], F16, tag="th")
                    dwt_h_piece(a, b, th_p)
                    if hc == 0:
                        # s_local=0, group 0: mirror (d1+d2)*y[0]+(d0+d3)*y[1]
                        nc.vector.tensor_scalar(y2_v[0:64, 0:1, :],
                                                y_v[0:64, 1:2, :],
                                                d0 + d3, None, ALU.mult)
                        nc.vector.scalar_tensor_tensor(
                            y2_v[0:64, 0:1, :], y_v[0:64, 0:1, :], d1 + d2,
                            y2_v[0:64, 0:1, :], ALU.mult, ALU.add)
                        nc.vector.tensor_tensor(pw_v[0:64, 0:1, :],
                                                y2_v[0:64, 0:1, 0::2],
                                                y2_v[0:64, 0:1, 1::2], ALU.max)
                    nc.vector.tensor_tensor(pw_v[:, a:b, :],
                                            y2_v[:, a:b, 0::2],
                                            y2_v[:, a:b, 1::2], ALU.max)


                # s_local=0, group 1 needs y rows 126,127 of group 0 (seam)
                seam = fp.tile([128, 2 * 128], F16, name="seam")
                nc.sync.dma_start(seam[64:128, :], y_t[0:64, 126*128:128*128])
                seam_v = seam[:].rearrange("p (h t) -> p h t", t=128)
                nc.vector.tensor_scalar(y2_v[64:128, 0:1, :],
                                        seam_v[64:128, 0:1, :], d3, None,
                                        ALU.mult)
                nc.vector.scalar_tensor_tensor(
                    y2_v[64:128, 0:1, :], seam_v[64:128, 1:2, :], d2,
                    y2_v[64:128, 0:1, :], ALU.mult, ALU.add)
                nc.vector.scalar_tensor_tensor(
                    y2_v[64:128, 0:1, :], y_v[64:128, 0:1, :], d1,
                    y2_v[64:128, 0:1, :], ALU.mult, ALU.add)
                nc.vector.scalar_tensor_tensor(
                    y2_v[64:128, 0:1, :], y_v[64:128, 1:2, :], d0,
                    y2_v[64:128, 0:1, :], ALU.mult, ALU.add)
                nc.vector.tensor_tensor(pw_v[64:128, 0:1, :],
                                        y2_v[64:128, 0:1, 0::2],
                                        y2_v[64:128, 0:1, 1::2], ALU.max)

            # pool-H (s pairs): group 0 (q 0..31) into padded conv1 input
            nc.vector.tensor_tensor(p1v[0:64, 1:33, 1:65],
                                    pw_v[0:64, 0::2, :], pw_v[0:64, 1::2, :],
                                    ALU.max)
            # group 1 (q 32..63) pooled on partitions 64:128, then DMA down
            nc.vector.tensor_tensor(pg1_v[64:128], pw_v[64:128, 0::2, :],
                                    pw_v[64:128, 1::2, :], ALU.max)
            nc.scalar.dma_start(p1v[0:64, 33:65, 1:65], pg1_v[64:128])
            front_pool.__exit__(None, None, None)
            xin_pool.__exit__(None, None, None)
            if stage <= 2:
                return

            # ---------------- conv1 (+BN1 stats) ----------------
            mid_pool = tc.tile_pool(name="mid", bufs=1)
            mp = mid_pool.__enter__()
            a1_sb = mp.tile([128, 4096], F16, name="a1_sb")
            junk = pp.tile([128, 512], F32, name="junk")
            s1b = pp.tile([128, 8], F32, name="s1b")
            s2b = pp.tile([128, 8], F32, name="s2b")
            a1v = a1_sb[:].rearrange("p (r q) -> p r q", q=64)

            with tc.tile_pool(name="psB", bufs=8, space="PSUM") as psB:
                ps_list = [psB.tile([128, 512], F32, tag="psB", name=f"c1ps{i}")
                           for i in range(8)]
                for ti in range(9):
                    ky, kx = divmod(ti, 3)
                    for ch in range(8):
                        p0 = ch * 8
                        rhs = p1v[0:64, p0+ky:p0+ky+8, kx:kx+64]
                        nc.tensor.matmul(ps_list[ch][:], w1t_sb[:, ti*128:(ti+1)*128],
                                         rhs, start=(ti == 0), stop=(ti == 8))
                for ch in range(8):
                    nc.vector.tensor_scalar(a1v[:, ch*8:ch*8+8, :], ps_list[ch][:],
                                            1.0, 0.0, ALU.mult, ALU.add,
                                            accum_out=s1b[:, ch:ch+1])
                    nc.scalar.activation(junk[:], ps_list[ch][:], AF.Square,
                                         accum_out=s2b[:, ch:ch+1])

            if stage <= 3:
                mid_pool.__exit__(None, None, None)
                return
            sc1, bi1 = _bn_coeffs(nc, pp, s1b, s2b, cc_bufs[1], rg, cnt,
                                  bnv["bn1g"], bnv["bn1b"], use_cc, tag=1)

            # BN1 + ReLU fused, written into padded conv2 input
            in2_pad = mp.tile([128, 66 * 66], F16, name="in2_pad")
            nc.vector.memset(in2_pad[:], 0.0)
            p2v = in2_pad[:].rearrange("p (r v) -> p r v", v=66)
            nc.scalar.activation(p2v[:, 1:65, 1:65], a1v, AF.Relu,
                                 bias=bi1[:], scale=sc1[:])

            if stage <= 4:
                mid_pool.__exit__(None, None, None)
                return
            # ---------------- conv2 (+BN2 stats) ----------------
            h2_sb = mp.tile([128, 4096], F16, name="h2_sb")
            h2v = h2_sb[:].rearrange("p (r q) -> p r q", q=64)
            s1c = pp.tile([128, 8], F32, name="s1c")
            s2c = pp.tile([128, 8], F32, name="s2c")
            with tc.tile_pool(name="psC", bufs=8, space="PSUM") as psC:
                ps_list = [psC.tile([128, 512], F32, tag="psC", name=f"c2ps{i}")
                           for i in range(8)]
                for ti in range(9):
                    ky, kx = divmod(ti, 3)
                    for ch in range(8):
                        p0 = ch * 8
                        rhs = p2v[:, p0+ky:p0+ky+8, kx:kx+64]
                        nc.tensor.matmul(ps_list[ch][:], w2t_sb[:, ti*128:(ti+1)*128],
                                         rhs, start=(ti == 0), stop=(ti == 8))
                for ch in range(8):
                    nc.vector.tensor_scalar(h2v[:, ch*8:ch*8+8, :], ps_list[ch][:],
                                            1.0, 0.0, ALU.mult, ALU.add,
                                            accum_out=s1c[:, ch:ch+1])
                    nc.scalar.activation(junk[:], ps_list[ch][:], AF.Square,
                                         accum_out=s2c[:, ch:ch+1])

            sc2, bi2 = _bn_coeffs(nc, pp, s1c, s2c, cc_bufs[2], rg, cnt,
                                  bnv["bn2g"], bnv["bn2b"], use_cc, tag=2)
            nc.scalar.activation(h2v, h2v, AF.Relu, bias=bi2[:], scale=sc2[:])

            if stage <= 5:
                mid_pool.__exit__(None, None, None)
                return
            # ---------------- deconv ----------------
            # computed in two q-halves so the DRAM round-trip that puts H on
            # partitions (scr2 write + dth read) pipelines with the second
            # half's matmuls
            dth = pp.tile([128, 64 * 128], F16, name="dth")
            dth_v = dth[:].rearrange("p (o w) -> p o w", w=128)
            scr2_h = scr2[:].rearrange("o h w -> h o w")
            d_sb = mp.tile([64, 128 * 128], F16, name="d_sb")
            dv = d_sb[:].rearrange("p (h w) -> p h w", w=128)
            with tc.tile_pool(name="psD", bufs=8, space="PSUM") as psD:
                for half in range(2):
                    for kl in range(4):
                        k, l = divmod(kl, 2)
                        for ch in range(4):
                            p0 = half * 32 + ch * 8
                            ps = psD.tile([64, 512], F32, tag="psD")
                            nc.tensor.matmul(ps[:], wdt_sb[:, kl*64:(kl+1)*64],
                                             h2v[:, p0:p0+8, :],
                                             start=True, stop=True)
                            dst = dv[:, 2*p0+k:2*p0+k+15:2, l::2]
                            if (kl * 4 + ch) % 2 == 0:
                                nc.vector.tensor_scalar(dst, ps[:], 1.0,
                                                        db_sb[:],
                                                        ALU.mult, ALU.add)
                            else:
                                nc.scalar.activation(dst, ps[:], AF.Identity,
                                                     bias=db_sb[:], scale=1.0)
                    h0 = half * 64
                    nc.sync.dma_start(scr2[:, h0:h0+64, :],
                                      dv[:, h0:h0+64, :])
                    nc.sync.dma_start(dth_v[h0:h0+64, 0:32, :],
                                      scr2_h[h0:h0+64, 0:32, :])
                    nc.scalar.dma_start(dth_v[h0:h0+64, 32:64, :],
                                        scr2_h[h0:h0+64, 32:64, :])
            mid_pool.__exit__(None, None, None)
            if stage <= 6:
                return

            # ---------------- IDWT-H on PE, IDWT-W on DVE ----------------
            with tc.tile_pool(name="psE", bufs=8, space="PSUM") as psE, \
                 tc.tile_pool(name="gpool", bufs=2) as gpool, \
                 tc.tile_pool(name="twpool", bufs=2) as twpool, \
                 tc.tile_pool(name="opool", bufs=2) as opool:
                for blk in range(2):
                    g_t = gpool.tile([127, 8192], F16, tag="g")
                    g_v = g_t[:].rearrange("p (o w) -> p o w", w=128)
                    for nch in range(16):
                        ps = psE.tile([127, 512], F32, tag="psE")
                        nc.tensor.matmul(ps[:], bhw_sb[:, blk*127:blk*127+127],
                                         dth[:, nch*512:(nch+1)*512],
                                         start=True, stop=True)
                        dst = g_t[:, nch*512:(nch+1)*512]
                        if nch % 2 == 0:
                            nc.vector.tensor_copy(dst, ps[:])
                        else:
                            nc.scalar.copy(dst, ps[:])
                    # o-halves double-buffered so block 1's IDWT-W runs while
                    # block 0's stores drain
                    for oh in range(2):
                        gh = g_v[:, oh*32:(oh+1)*32, :]
                        o_t = opool.tile([127, 32 * 254], F32, tag="o")
                        o_v = o_t[:].rearrange("p (o w) -> p o w", w=254)
                        tw = twpool.tile([127, 32 * 128], F16, tag="tw")
                        tw_v = tw[:].rearrange("p (o w) -> p o w", w=128)
                        nc.vector.tensor_scalar(tw_v, gh, float(REC[2]), None,
                                                ALU.mult)
                        nc.vector.scalar_tensor_tensor(
                            o_v[:, :, 0:253:2], gh[:, :, 1:128], float(REC[0]),
                            tw_v[:, :, 0:127], ALU.mult, ALU.add)
                        nc.vector.tensor_scalar(tw_v, gh, float(REC[3]), None,
                                                ALU.mult)
                        nc.vector.scalar_tensor_tensor(
                            o_v[:, :, 1:254:2], gh[:, :, 1:128], float(REC[1]),
                            tw_v[:, :, 0:127], ALU.mult, ALU.add)
                        for i in range(8):
                            oi = blk * 8 + i
                            h0, sz = OUT_SPLITS[oi]
                            p0 = h0 - blk * 127
                            eng = nc.sync if (i + oh) % 2 == 0 else nc.scalar
                            eng.dma_start(out_ds[oi][:, oh*32:(oh+1)*32, :],
                                          o_v[p0:p0+sz])

        _body()
    nc.compile()
    return nc


def _bn_coeffs(nc, pp, s1b, s2b, cc_pair, rg, cnt, g_sb, b_sb, use_cc, tag):
    """Reduce per-chunk sums, AllReduce across cores, return (scale, bias) [128,1]."""
    ALU = mybir.AluOpType
    sl = pp.tile([128, 2], F32, name=f"bn{tag}_sl")
    nc.vector.tensor_reduce(sl[:, 0:1], s1b[:], mybir.AxisListType.X, ALU.add)
    nc.vector.tensor_reduce(sl[:, 1:2], s2b[:], mybir.AxisListType.X, ALU.add)
    cc_in, cc_out = cc_pair
    sg = pp.tile([128, 2], F32, name=f"bn{tag}_sg")
    if use_cc:
        nc.sync.dma_start(cc_in[:], sl[:])
        nc.gpsimd.collective_compute(
            "AllReduce", ALU.add, replica_groups=rg,
            ins=[cc_in[:]], outs=[cc_out[:]])
        nc.sync.dma_start(sg[:], cc_out[:])
    else:
        nc.vector.tensor_copy(sg[:], sl[:])

    m = pp.tile([128, 1], F32, name=f"bn{tag}_m")
    vpe = pp.tile([128, 1], F32, name=f"bn{tag}_v")
    t0 = pp.tile([128, 1], F32, name=f"bn{tag}_t0")
    nc.vector.tensor_scalar(m[:], sg[:, 0:1], 1.0 / cnt, None, ALU.mult)
    nc.vector.tensor_tensor(t0[:], m[:], m[:], ALU.mult)          # m^2
    nc.vector.tensor_scalar(vpe[:], sg[:, 1:2], 1.0 / cnt, float(EPS), ALU.mult,
                            ALU.add)                              # E[x^2]+eps
    nc.vector.tensor_tensor(vpe[:], vpe[:], t0[:], ALU.subtract)  # var+eps
    # rsqrt with one Newton step (ACT Sqrt is low-precision)
    s0 = pp.tile([128, 1], F32, name=f"bn{tag}_s0")
    y0 = pp.tile([128, 1], F32, name=f"bn{tag}_y0")
    nc.scalar.activation(s0[:], vpe[:], mybir.ActivationFunctionType.Sqrt)
    nc.vector.reciprocal(y0[:], s0[:])
    t1 = pp.tile([128, 1], F32, name=f"bn{tag}_t1")
    nc.vector.tensor_tensor(t1[:], y0[:], y0[:], ALU.mult)
    nc.vector.tensor_tensor(t1[:], t1[:], vpe[:], ALU.mult)
    nc.vector.tensor_scalar(t1[:], t1[:], -0.5, 1.5, ALU.mult, ALU.add)
    nc.vector.tensor_tensor(y0[:], y0[:], t1[:], ALU.mult)        # refined rsqrt
    sc = pp.tile([128, 1], F32, name=f"bn{tag}_sc")
    bi = pp.tile([128, 1], F32, name=f"bn{tag}_bi")
    nc.vector.tensor_tensor(sc[:], y0[:], g_sb[:], ALU.mult)
    nc.vector.tensor_tensor(t0[:], m[:], sc[:], ALU.mult)
    nc.vector.tensor_tensor(bi[:], b_sb[:], t0[:], ALU.subtract)
    return sc, bi


# ---------------------------------------------------------------- entry point
_CACHE = {}


def kernel(x, conv1_w, conv1_b, bn1_g, bn1_b, conv2_w, conv2_b, bn2_g, bn2_b,
           deconv_w, deconv_b):
    world = N_CORES
    if "nc" not in _CACHE:
        _CACHE["nc"] = build_nc(world)
    nc = _CACHE["nc"]

    consts = pack_consts(np.asarray(conv1_w), np.asarray(conv2_w),
                         np.asarray(deconv_w), np.asarray(deconv_b),
                         np.asarray(bn1_g), np.asarray(bn1_b),
                         np.asarray(bn2_g), np.asarray(bn2_b))
    x = np.asarray(x)
    in_maps = []
    for n in range(world):
        m = {"x": np.ascontiguousarray(x[n])}
        m.update(consts)
        in_maps.append(m)

    res = run_bass_kernel_spmd(
        nc, in_maps, core_ids=list(range(world)),
        trace=bool(int(os.environ.get("WK_TRACE", "0"))))
    out = np.stack(
        [np.concatenate([r[f"out{i}"] for i in range(16)], axis=0).transpose(1, 0, 2)
         for r in res.results], axis=0)
    _CACHE["last_perf"] = res
    return out


# revision 51
# speedup vs baseline: 1.0266x; 1.0266x over previous
"""Trainium2 Bass kernel for nn_Center2D (DWT -> pool -> conv-BN-ReLU x2 -> deconv -> IDWT).

Self-contained: hardcodes shapes from the problem spec.
Sharding: pure data parallel, batch dim (8) across 8 cores; BN batch stats
synchronized with a tiny AllReduce (2x128 floats) per BN layer.

Layout strategy per core (one sample):
  front: x loaded contiguously as [(h-half, c) -> 128 partitions, 128 h, 256 w]
         (32KB descriptors, sprays across all 16 SDMA engines), DWT-W and
         DWT-H as 4-tap FIR on DVE (strided slices along free dims), pool on
         DVE, one small SBUF->SBUF DMA to consolidate h-halves for conv1,
  mid:   conv1 as 9 matmuls (64-part contraction), conv2 as 9 K-packed PE
         matmuls per output chunk, BN stats via accum_out during PSUM
         evacuation, tiny AllReduce (warmed up by a dummy collective at t=0),
         BN+ReLU fused into one ACT op,
  back:  deconv as 4 PE matmuls, DRAM round-trip to put H on partitions,
         PE matmul for IDWT-H (banded matrix BH), DVE taps for final IDWT-W,
         output stores split across 16 ExternalOutput tensors (stores to one
         ExternalOutput pin all descriptors of a call onto one SDMA engine).
"""

import os
import numpy as np

import concourse.bass as bass
import concourse.bacc as bacc
import concourse.tile as tile
from concourse import mybir
from concourse.bass_utils import run_bass_kernel_spmd

F32 = mybir.dt.float32
F16 = mybir.dt.float16
AF = mybir.ActivationFunctionType
ALU = mybir.AluOpType

REC = np.array([0.48296291314469025, 0.8365163037378079,
                0.22414386804185735, -0.12940952255092145], dtype=np.float64)
DEC = REC[::-1].copy()

N_CORES = int(os.environ.get("WK_CORES", "8"))
EPS = 1e-5


# ---------------------------------------------------------------- host consts
def build_BH():
    """IDWT along one axis as a dense [128, 254] matrix."""
    B = np.zeros((128, 254), dtype=np.float64)
    for t in range(127):
        B[t,   2*t] += REC[2]
        B[t+1, 2*t] += REC[0]
        B[t,   2*t+1] += REC[3]
        B[t+1, 2*t+1] += REC[1]
    return B.astype(np.float32)


def pack_consts(conv1_w, conv2_w, deconv_w, deconv_b, bn1_g, bn1_b, bn2_g, bn2_b):
    bhw = build_BH().astype(np.float16)          # [128, 254]

    w1t = np.zeros((64, 9 * 128), np.float16)    # rows ci, cols (ky,kx,co)
    for ky in range(3):
        for kx in range(3):
            w1t[:, (ky*3+kx)*128:(ky*3+kx+1)*128] = conv1_w[:, :, ky, kx].T

    w2t = np.zeros((128, 9 * 128), np.float16)
    for ky in range(3):
        for kx in range(3):
            w2t[:, (ky*3+kx)*128:(ky*3+kx+1)*128] = conv2_w[:, :, ky, kx].T

    wdt = np.zeros((128, 4 * 64), np.float16)    # [ci, (k,l,o)]
    for k in range(2):
        for l in range(2):
            wdt[:, (k*2+l)*64:(k*2+l+1)*64] = deconv_w[:, :, k, l]

    return {
        "BHW": bhw,
        "w1t": w1t,
        "w2t": w2t,
        "wdt": wdt,
        "db": deconv_b.reshape(64, 1).astype(np.float32),
        "bn1g": bn1_g.reshape(128, 1).astype(np.float32),
        "bn1b": bn1_b.reshape(128, 1).astype(np.float32),
        "bn2g": bn2_g.reshape(128, 1).astype(np.float32),
        "bn2b": bn2_b.reshape(128, 1).astype(np.float32),
    }


# ---------------------------------------------------------------- bass kernel
def build_nc(world=N_CORES, stage=None):
    if stage is None:
        stage = int(os.environ.get("WK_STAGE", "99"))
    nc = bacc.Bacc("TRN2", target_bir_lowering=False)
    use_cc = world > 1

    x = nc.dram_tensor("x", (64, 256, 256), F32, kind="ExternalInput")
    bhw_d = nc.dram_tensor("BHW", (128, 254), F16, kind="ExternalInput")
    w1t_d = nc.dram_tensor("w1t", (64, 1152), F16, kind="ExternalInput")
    w2t_d = nc.dram_tensor("w2t", (128, 1152), F16, kind="ExternalInput")
    wdt_d = nc.dram_tensor("wdt", (128, 256), F16, kind="ExternalInput")
    db_d = nc.dram_tensor("db", (64, 1), F32, kind="ExternalInput")
    bn_vecs = {n: nc.dram_tensor(n, (128, 1), F32, kind="ExternalInput")
               for n in ("bn1g", "bn1b", "bn2g", "bn2b")}
    # 16 separate output tensors (h-slices): writes to an ExternalOutput pin
    # all descriptors of a call onto one SDMA engine, so split into 16 calls
    OUT_SPLITS = []
    h0 = 0
    for i in range(16):
        sz = 16 if (i % 8) != 7 else 15
        OUT_SPLITS.append((h0, sz))
        h0 += sz
    out_ds = [nc.dram_tensor(f"out{i}", (sz, 64, 254), F32, kind="ExternalOutput")
              for i, (_, sz) in enumerate(OUT_SPLITS)]

    scr2 = nc.dram_tensor("scr2", (64, 128, 128), F16, kind="Internal")
    cc_bufs = []
    for i in (0, 1, 2):
        cc_bufs.append((
            nc.dram_tensor(f"bn{i}_in", (128, 2), F32, kind="Internal"),
            nc.dram_tensor(f"bn{i}_out", (128, 2), F32, kind="Internal",
                           addr_space="Shared"),
        ))
    rg = [list(range(world))]
    cnt = float(world * 64 * 64)

    with tile.TileContext(nc) as tc, \
         tc.tile_pool(name="persist", bufs=1) as pp:
        def _body():
            # warmup collective: absorbs the ~11us first-call ncfw setup so
            # the BN1 AllReduce doesn't pay it; runs under the x load
            if use_cc:
                nc.gpsimd.collective_compute(
                    "AllReduce", ALU.add, replica_groups=rg,
                    ins=[cc_bufs[0][0][:]], outs=[cc_bufs[0][1][:]])

            # ---------------- x chunk 0 load first: it heads the whole
            # dependency graph, so it must not queue behind the const DMAs
            xg = x[:].rearrange("c (g r) w -> g c (r w)", g=2)  # [2,64,128*256]
            xin_pool = tc.tile_pool(name="xin", bufs=2)
            xin = xin_pool.__enter__()
            xc0 = xin.tile([128, 32 * 256], F32, tag="xc")
            nc.sync.dma_start(xc0[0:64], xg[0, :, 0:8192])
            nc.scalar.dma_start(xc0[64:128], xg[1, :, 0:8192])

            # ---------------- consts to SBUF
            bhw_sb = pp.tile([128, 254], F16, name="bhw_sb")
            nc.sync.dma_start(bhw_sb[:], bhw_d[:])
            w1t_sb = pp.tile([64, 1152], F16, name="w1t_sb")
            nc.sync.dma_start(w1t_sb[:], w1t_d[:])
            w2t_sb = pp.tile([128, 1152], F16, name="w2t_sb")
            nc.sync.dma_start(w2t_sb[:], w2t_d[:])
            wdt_sb = pp.tile([128, 256], F16, name="wdt_sb")
            nc.sync.dma_start(wdt_sb[:], wdt_d[:])
            db_sb = pp.tile([64, 1], F32, name="db_sb")
            nc.sync.dma_start(db_sb[:], db_d[:])
            bnv = {}
            for n, d in bn_vecs.items():
                t = pp.tile([128, 1], F32, name=f"{n}_sb")
                nc.sync.dma_start(t[:], d[:])
                bnv[n] = t

            # conv1 input (lives past the front scope), 64 partitions
            in1_pad = pp.tile([64, 66 * 66], F16, name="in1_pad")
            nc.vector.memset(in1_pad[:], 0.0)
            p1v = in1_pad[:].rearrange("p (r v) -> p r v", v=66)

            d0, d1, d2, d3 = (float(DEC[0]), float(DEC[1]),
                              float(DEC[2]), float(DEC[3]))

            # ---------------- front: all-DVE DWT + pool
            # x viewed as [(g c) -> 128 partitions, h_local, w]; partition
            # group g=0 holds h 0:128, g=1 holds h 128:256 of channel c
            front_pool = tc.tile_pool(name="front", bufs=1)
            fp = front_pool.__enter__()

            y_t = fp.tile([128, 128 * 128], F16, name="y_t")    # DWT-W out
            y_v = y_t[:].rearrange("p (h t) -> p h t", t=128)
            y2_t = fp.tile([128, 64 * 128], F16, name="y2_t")   # DWT-H out
            y2_v = y2_t[:].rearrange("p (s t) -> p s t", t=128)
            pw_t = fp.tile([128, 64 * 64], F16, name="pw_t")    # pool-W out
            pw_v = pw_t[:].rearrange("p (s u) -> p s u", u=64)
            pg1 = fp.tile([128, 32 * 64], F16, name="pg1")      # pool-H grp 1
            pg1_v = pg1[:].rearrange("p (s u) -> p s u", u=64)

            def dwt_h_piece(a, b, th_p):
                """y2[s] = d3 y[2s-2] + d2 y[2s-1] + d1 y[2s] + d0 y[2s+1],
                s_local in [a, b) (a >= 1), both partition groups at once."""
                tv2 = th_p[:].rearrange("p (s t) -> p s t", t=128)[:, 0:b-a, :]
                nc.vector.tensor_scalar(tv2, y_v[:, 2*a-2:2*b-3:2, :], d3,
                                        None, ALU.mult)
                nc.vector.scalar_tensor_tensor(tv2, y_v[:, 2*a-1:2*b-2:2, :],
                                               d2, tv2, ALU.mult, ALU.add)
                nc.vector.scalar_tensor_tensor(tv2, y_v[:, 2*a:2*b-1:2, :],
                                               d1, tv2, ALU.mult, ALU.add)
                nc.vector.scalar_tensor_tensor(y2_v[:, a:b, :],
                                               y_v[:, 2*a+1:2*b:2, :],
                                               d0, tv2, ALU.mult, ALU.add)

            with tc.tile_pool(name="twp", bufs=2) as twp, \
                 tc.tile_pool(name="thp", bufs=2) as thp:
                for hc in range(4):          # h-chunks of 32 rows
                    if hc == 0:
                        xc = xc0
                    else:
                        xc = xin.tile([128, 32 * 256], F32, tag="xc")
                        # one call per h-half: a [64, 8192] 2-dim AP sprays
                        # per-partition; a 3-dim [2, 64, 8192] sprays by dim0
                        # and serializes onto 2 engines
                        nc.sync.dma_start(xc[0:64], xg[0, :, hc*8192:(hc+1)*8192])
                        nc.scalar.dma_start(xc[64:128],
                                            xg[1, :, hc*8192:(hc+1)*8192])
                    xv = xc[:].rearrange("p (h w) -> p h w", w=256)
                    yc = y_v[:, hc*32:(hc+1)*32, :]
                    tw = twp.tile([128, 32 * 127], F16, tag="tw")
                    tv = tw[:].rearrange("p (h t) -> p h t", t=127)
                    # DWT-W main taps t=1..127
                    nc.vector.tensor_scalar(tv, xv[:, :, 0:253:2], d3, None,
                                            ALU.mult)
                    nc.vector.scalar_tensor_tensor(tv, xv[:, :, 1:254:2], d2,
                                                   tv, ALU.mult, ALU.add)
                    nc.vector.scalar_tensor_tensor(tv, xv[:, :, 2:255:2], d1,
                                                   tv, ALU.mult, ALU.add)
                    nc.vector.scalar_tensor_tensor(yc[:, :, 1:128],
                                                   xv[:, :, 3:256:2], d0,
                                                   tv, ALU.mult, ALU.add)
                    # t=0 mirror: (d1+d2)*x0 + (d0+d3)*x1
                    nc.vector.tensor_scalar(yc[:, :, 0:1], xv[:, :, 1:2],
                                            d0 + d3, None, ALU.mult)
                    nc.vector.scalar_tensor_tensor(yc[:, :, 0:1],
                                                   xv[:, :, 0:1], d1 + d2,
                                                   yc[:, :, 0:1],
                                                   ALU.mult, ALU.add)
                    # DWT-H piece for the s-range this chunk completes
                    # (chunk hc covers h <= 32*hc+31 -> s <= 16*hc+15),
                    # then pool-W for those s rows; both hide under the
                    # next chunk's load
                    a = max(1, 16 * hc)
                    b = 16 * hc + 16
                    th_p = thp.tile([128, 16 * 128], F16, tag="th")
                    dwt_h_piece(a, b, th_p)
                    if hc == 0:
                        # s_local=0, group 0: mirror (d1+d2)*y[0]+(d0+d3)*y[1]
                        nc.vector.tensor_scalar(y2_v[0:64, 0:1, :],
                                                y_v[0:64, 1:2, :],
                                                d0 + d3, None, ALU.mult)
                        nc.vector.scalar_tensor_tensor(
                            y2_v[0:64, 0:1, :], y_v[0:64, 0:1, :], d1 + d2,
                            y2_v[0:64, 0:1, :], ALU.mult, ALU.add)
                        nc.vector.tensor_tensor(pw_v[0:64, 0:1, :],
                                                y2_v[0:64, 0:1, 0::2],
                                                y2_v[0:64, 0:1, 1::2], ALU.max)
                    nc.vector.tensor_tensor(pw_v[:, a:b, :],
                                            y2_v[:, a:b, 0::2],
                                            y2_v[:, a:b, 1::2], ALU.max)


                # s_local=0, group 1 needs y rows 126,127 of group 0 (seam)
                seam = fp.tile([128, 2 * 128], F16, name="seam")
                nc.sync.dma_start(seam[64:128, :], y_t[0:64, 126*128:128*128])
                seam_v = seam[:].rearrange("p (h t) -> p h t", t=128)
                nc.vector.tensor_scalar(y2_v[64:128, 0:1, :],
                                        seam_v[64:128, 0:1, :], d3, None,
                                        ALU.mult)
                nc.vector.scalar_tensor_tensor(
                    y2_v[64:128, 0:1, :], seam_v[64:128, 1:2, :], d2,
                    y2_v[64:128, 0:1, :], ALU.mult, ALU.add)
                nc.vector.scalar_tensor_tensor(
                    y2_v[64:128, 0:1, :], y_v[64:128, 0:1, :], d1,
                    y2_v[64:128, 0:1, :], ALU.mult, ALU.add)
                nc.vector.scalar_tensor_tensor(
                    y2_v[64:128, 0:1, :], y_v[64:128, 1:2, :], d0,
                    y2_v[64:128, 0:1, :], ALU.mult, ALU.add)
                nc.vector.tensor_tensor(pw_v[64:128, 0:1, :],
                                        y2_v[64:128, 0:1, 0::2],
                                        y2_v[64:128, 0:1, 1::2], ALU.max)

            # pool-H (s pairs): group 0 (q 0..31) into padded conv1 input
            nc.vector.tensor_tensor(p1v[0:64, 1:33, 1:65],
                                    pw_v[0:64, 0::2, :], pw_v[0:64, 1::2, :],
                                    ALU.max)
            # group 1 (q 32..63) pooled on partitions 64:128, then DMA down
            nc.vector.tensor_tensor(pg1_v[64:128], pw_v[64:128, 0::2, :],
                                    pw_v[64:128, 1::2, :], ALU.max)
            nc.scalar.dma_start(p1v[0:64, 33:65, 1:65], pg1_v[64:128])
            front_pool.__exit__(None, None, None)
            xin_pool.__exit__(None, None, None)
            if stage <= 2:
                return

            # ---------------- conv1 (+BN1 stats) ----------------
            mid_pool = tc.tile_pool(name="mid", bufs=1)
            mp = mid_pool.__enter__()
            a1_sb = mp.tile([128, 4096], F16, name="a1_sb")
            junk = pp.tile([128, 512], F32, name="junk")
            s1b = pp.tile([128, 8], F32, name="s1b")
            s2b = pp.tile([128, 8], F32, name="s2b")
            a1v = a1_sb[:].rearrange("p (r q) -> p r q", q=64)

            with tc.tile_pool(name="psB", bufs=8, space="PSUM") as psB:
                ps_list = [psB.tile([128, 512], F32, tag="psB", name=f"c1ps{i}")
                           for i in range(8)]
                for ti in range(9):
                    ky, kx = divmod(ti, 3)
                    for ch in range(8):
                        p0 = ch * 8
                        rhs = p1v[0:64, p0+ky:p0+ky+8, kx:kx+64]
                        nc.tensor.matmul(ps_list[ch][:], w1t_sb[:, ti*128:(ti+1)*128],
                                         rhs, start=(ti == 0), stop=(ti == 8))
                for ch in range(8):
                    nc.vector.tensor_scalar(a1v[:, ch*8:ch*8+8, :], ps_list[ch][:],
                                            1.0, 0.0, ALU.mult, ALU.add,
                                            accum_out=s1b[:, ch:ch+1])
                    nc.scalar.activation(junk[:], ps_list[ch][:], AF.Square,
                                         accum_out=s2b[:, ch:ch+1])

            if stage <= 3:
                mid_pool.__exit__(None, None, None)
                return
            sc1, bi1 = _bn_coeffs(nc, pp, s1b, s2b, cc_bufs[1], rg, cnt,
                                  bnv["bn1g"], bnv["bn1b"], use_cc, tag=1)

            # BN1 + ReLU fused, written into padded conv2 input
            in2_pad = mp.tile([128, 66 * 66], F16, name="in2_pad")
            nc.vector.memset(in2_pad[:], 0.0)
            p2v = in2_pad[:].rearrange("p (r v) -> p r v", v=66)
            nc.scalar.activation(p2v[:, 1:65, 1:65], a1v, AF.Relu,
                                 bias=bi1[:], scale=sc1[:])

            if stage <= 4:
                mid_pool.__exit__(None, None, None)
                return
            # ---------------- conv2 (+BN2 stats) ----------------
            h2_sb = mp.tile([128, 4096], F16, name="h2_sb")
            h2v = h2_sb[:].rearrange("p (r q) -> p r q", q=64)
            s1c = pp.tile([128, 8], F32, name="s1c")
            s2c = pp.tile([128, 8], F32, name="s2c")
            with tc.tile_pool(name="psC", bufs=8, space="PSUM") as psC:
                ps_list = [psC.tile([128, 512], F32, tag="psC", name=f"c2ps{i}")
                           for i in range(8)]
                for ti in range(9):
                    ky, kx = divmod(ti, 3)
                    for ch in range(8):
                        p0 = ch * 8
                        rhs = p2v[:, p0+ky:p0+ky+8, kx:kx+64]
                        nc.tensor.matmul(ps_list[ch][:], w2t_sb[:, ti*128:(ti+1)*128],
                                         rhs, start=(ti == 0), stop=(ti == 8))
                for ch in range(8):
                    nc.vector.tensor_scalar(h2v[:, ch*8:ch*8+8, :], ps_list[ch][:],
                                            1.0, 0.0, ALU.mult, ALU.add,
                                            accum_out=s1c[:, ch:ch+1])
                    nc.scalar.activation(junk[:], ps_list[ch][:], AF.Square,
                                         accum_out=s2c[:, ch:ch+1])

            sc2, bi2 = _bn_coeffs(nc, pp, s1c, s2c, cc_bufs[2], rg, cnt,
                                  bnv["bn2g"], bnv["bn2b"], use_cc, tag=2)
            nc.scalar.activation(h2v, h2v, AF.Relu, bias=bi2[:], scale=sc2[:])

            if stage <= 5:
                mid_pool.__exit__(None, None, None)
                return
            # ---------------- deconv ----------------
            # computed in two q-halves so the DRAM round-trip that puts H on
            # partitions (scr2 write + dth read) pipelines with the second
            # half's matmuls
            dth = pp.tile([128, 64 * 128], F16, name="dth")
            dth_v = dth[:].rearrange("p (o w) -> p o w", w=128)
            scr2_h = scr2[:].rearrange("o h w -> h o w")
            d_sb = mp.tile([64, 128 * 128], F16, name="d_sb")
            dv = d_sb[:].rearrange("p (h w) -> p h w", w=128)
            with tc.tile_pool(name="psD", bufs=8, space="PSUM") as psD:
                for half in range(2):
                    for kl in range(4):
                        k, l = divmod(kl, 2)
                        for ch in range(4):
                            p0 = half * 32 + ch * 8
                            ps = psD.tile([64, 512], F32, tag="psD")
                            nc.tensor.matmul(ps[:], wdt_sb[:, kl*64:(kl+1)*64],
                                             h2v[:, p0:p0+8, :],
                                             start=True, stop=True)
                            dst = dv[:, 2*p0+k:2*p0+k+15:2, l::2]
                            if (kl * 4 + ch) % 2 == 0:
                                nc.vector.tensor_scalar(dst, ps[:], 1.0,
                                                        db_sb[:],
                                                        ALU.mult, ALU.add)
                            else:
                                nc.scalar.activation(dst, ps[:], AF.Identity,
                                                     bias=db_sb[:], scale=1.0)
                    h0 = half * 64
                    nc.sync.dma_start(scr2[:, h0:h0+64, :],
                                      dv[:, h0:h0+64, :])
                    nc.sync.dma_start(dth_v[h0:h0+64, 0:32, :],
                                      scr2_h[h0:h0+64, 0:32, :])
                    nc.scalar.dma_start(dth_v[h0:h0+64, 32:64, :],
                                        scr2_h[h0:h0+64, 32:64, :])
            mid_pool.__exit__(None, None, None)
            if stage <= 6:
                return

            # ---------------- IDWT-H on PE, IDWT-W on DVE ----------------
            with tc.tile_pool(name="psE", bufs=8, space="PSUM") as psE, \
                 tc.tile_pool(name="gpool", bufs=2) as gpool, \
                 tc.tile_pool(name="twpool", bufs=2) as twpool, \
                 tc.tile_pool(name="opool", bufs=2) as opool:
                for blk in range(2):
                    g_t = gpool.tile([127, 8192], F16, tag="g")
                    g_v = g_t[:].rearrange("p (o w) -> p o w", w=128)
                    for nch in range(16):
                        ps = psE.tile([127, 512], F32, tag="psE")
                        nc.tensor.matmul(ps[:], bhw_sb[:, blk*127:blk*127+127],
                                         dth[:, nch*512:(nch+1)*512],
                                         start=True, stop=True)
                        dst = g_t[:, nch*512:(nch+1)*512]
                        if nch % 2 == 0:
                            nc.vector.tensor_copy(dst, ps[:])
                        else:
                            nc.scalar.copy(dst, ps[:])
                    # o-halves double-buffered so block 1's IDWT-W runs while
                    # block 0's stores drain
                    for oh in range(2):
                        gh = g_v[:, oh*32:(oh+1)*32, :]
                        o_t = opool.tile([127, 32 * 254], F32, tag="o")
                        o_v = o_t[:].rearrange("p (o w) -> p o w", w=254)
                        tw = twpool.tile([127, 32 * 128], F16, tag="tw")
                        tw_v = tw[:].rearrange("p (o w) -> p o w", w=128)
                        nc.vector.tensor_scalar(tw_v, gh, float(REC[2]), None,
                                                ALU.mult)
                        nc.vector.scalar_tensor_tensor(
                            o_v[:, :, 0:253:2], gh[:, :, 1:128], float(REC[0]),
                            tw_v[:, :, 0:127], ALU.mult, ALU.add)
                        nc.vector.tensor_scalar(tw_v, gh, float(REC[3]), None,
                                                ALU.mult)
                        nc.vector.scalar_tensor_tensor(
                            o_v[:, :, 1:254:2], gh[:, :, 1:128], float(REC[1]),
                            tw_v[:, :, 0:127], ALU.mult, ALU.add)
                        for i in range(8):
                            oi = blk * 8 + i
                            h0, sz = OUT_SPLITS[oi]
                            p0 = h0 - blk * 127
                            eng = nc.sync if (i + oh) % 2 == 0 else nc.scalar
                            eng.dma_start(out_ds[oi][:, oh*32:(oh+1)*32, :],
                                          o_v[p0:p0+sz])

        _body()
    nc.compile()
    return nc


def _bn_coeffs(nc, pp, s1b, s2b, cc_pair, rg, cnt, g_sb, b_sb, use_cc, tag):
    """Reduce per-chunk sums, AllReduce across cores, return (scale, bias) [128,1]."""
    ALU = mybir.AluOpType
    sl = pp.tile([128, 2], F32, name=f"bn{tag}_sl")
    nc.vector.tensor_reduce(sl[:, 0:1], s1b[:], mybir.AxisListType.X, ALU.add)
    nc.vector.tensor_reduce(sl[:, 1:2], s2b[:], mybir.AxisListType.X, ALU.add)
    cc_in, cc_out = cc_pair
    sg = pp.tile([128, 2], F32, name=f"bn{tag}_sg")
    if use_cc:
        nc.sync.dma_start(cc_in[:], sl[:])
        nc.gpsimd.collective_compute(
            "AllReduce", ALU.add, replica_groups=rg,
            ins=[cc_in[:]], outs=[cc_out[:]])
        nc.sync.dma_start(sg[:], cc_out[:])
    else:
        nc.vector.tensor_copy(sg[:], sl[:])

    m = pp.tile([128, 1], F32, name=f"bn{tag}_m")
    vpe = pp.tile([128, 1], F32, name=f"bn{tag}_v")
    t0 = pp.tile([128, 1], F32, name=f"bn{tag}_t0")
    nc.vector.tensor_scalar(m[:], sg[:, 0:1], 1.0 / cnt, None, ALU.mult)
    nc.vector.tensor_tensor(t0[:], m[:], m[:], ALU.mult)          # m^2
    nc.vector.tensor_scalar(vpe[:], sg[:, 1:2], 1.0 / cnt, float(EPS), ALU.mult,
                            ALU.add)                              # E[x^2]+eps
    nc.vector.tensor_tensor(vpe[:], vpe[:], t0[:], ALU.subtract)  # var+eps
    # rsqrt with one Newton step (ACT Sqrt is low-precision)
    s0 = pp.tile([128, 1], F32, name=f"bn{tag}_s0")
    y0 = pp.tile([128, 1], F32, name=f"bn{tag}_y0")
    nc.scalar.activation(s0[:], vpe[:], mybir.ActivationFunctionType.Sqrt)
    nc.vector.reciprocal(y0[:], s0[:])
    t1 = pp.tile([128, 1], F32, name=f"bn{tag}_t1")
    nc.vector.tensor_tensor(t1[:], y0[:], y0[:], ALU.mult)
    nc.vector.tensor_tensor(t1[:], t1[:], vpe[:], ALU.mult)
    nc.vector.tensor_scalar(t1[:], t1[:], -0.5, 1.5, ALU.mult, ALU.add)
    nc.vector.tensor_tensor(y0[:], y0[:], t1[:], ALU.mult)        # refined rsqrt
    sc = pp.tile([128, 1], F32, name=f"bn{tag}_sc")
    bi = pp.tile([128, 1], F32, name=f"bn{tag}_bi")
    nc.vector.tensor_tensor(sc[:], y0[:], g_sb[:], ALU.mult)
    nc.vector.tensor_tensor(t0[:], m[:], sc[:], ALU.mult)
    nc.vector.tensor_tensor(bi[:], b_sb[:], t0[:], ALU.subtract)
    return sc, bi


# ---------------------------------------------------------------- entry point
_CACHE = {}


def kernel(x, conv1_w, conv1_b, bn1_g, bn1_b, conv2_w, conv2_b, bn2_g, bn2_b,
           deconv_w, deconv_b):
    world = N_CORES
    if "nc" not in _CACHE:
        _CACHE["nc"] = build_nc(world)
    nc = _CACHE["nc"]

    consts = pack_consts(np.asarray(conv1_w), np.asarray(conv2_w),
                         np.asarray(deconv_w), np.asarray(deconv_b),
                         np.asarray(bn1_g), np.asarray(bn1_b),
                         np.asarray(bn2_g), np.asarray(bn2_b))
    x = np.asarray(x)
    in_maps = []
    for n in range(world):
        m = {"x": np.ascontiguousarray(x[n])}
        m.update(consts)
        in_maps.append(m)

    res = run_bass_kernel_spmd(
        nc, in_maps, core_ids=list(range(world)),
        trace=bool(int(os.environ.get("WK_TRACE", "0"))))
    out = np.stack(
        [np.concatenate([r[f"out{i}"] for i in range(16)], axis=0).transpose(1, 0, 2)
         for r in res.results], axis=0)
    _CACHE["last_perf"] = res
    return out
